# revision 1
# baseline (speedup 1.0000x reference)
"""Trainium2 Bass kernel for nn_NormalizedDistanceLoss.

Math: for x in R^{N x D}, with sq_i = ||x_i||^2, the strict-upper-triangle
sum of pairwise squared distances collapses algebraically:

    sum_{i<j} (sq_i + sq_j - 2 x_i.x_j) = N * S - ||s||^2

where S = sum_i sq_i and s = sum_i x_i (column sums).  So the loss

    loss = sum_masked_dist / (sqrt(max_i sq_i) * N(N-1)/2)

needs only one pass over x: per-row squared norms (for S and the max)
and column sums (for s).  Each of the 8 cores reduces its 1024-row block;
the host combines tiny per-core partials (a few KB per core).

Per-core device kernel (block = 1024 x 512 f32):
  - SBUF layout (128, 8, 512): partition p holds DRAM rows p*8..p*8+7
    (16KB contiguous per partition).  4 chunked DMAs (2 row-tiles each)
    split across BOTH HWDGE rings (sync + scalar) so transfers overlap.
  - Row squared norms: one fused square+row-sum op per 512-wide tile;
    ACT (Square activation + accum_out) for even tiles, DVE
    (scalar_tensor_tensor + accum_out) for odd tiles.
  - Column sums: DVE adds each tile pair into a bf16 pair tile; the
    otherwise-idle PE contracts the 128 partitions with a ones-vector
    matmul, accumulating all pairs in one PSUM bank.  bf16 pair rounding
    perturbs the final loss at ~1e-8 relative - far below fp32 noise.
"""

import sys

if "/opt/trn_rl_repo" not in sys.path:
    sys.path.insert(0, "/opt/trn_rl_repo")

import numpy as np

import concourse.bass as bass
import concourse.tile as tile
from concourse import bacc, mybir

N = 8192
D = 512
NCORES = 8
ROWS = N // NCORES  # 1024 rows per core
P = 128
T = ROWS // P  # 8 row-tiles of 512
NCHUNKS = 4
TPC = T // NCHUNKS  # row-tiles per DMA chunk (2)

_nc_cache = []


def _build_nc():
    f32 = mybir.dt.float32
    bf16 = mybir.dt.bfloat16
    nc = bacc.Bacc(
        "TRN2",
        target_bir_lowering=False,
        debug=False,
        num_devices=NCORES,
    )
    x_dram = nc.dram_tensor("x_blk", [ROWS, D], f32, kind="ExternalInput")
    rowsq_dram = nc.dram_tensor("rowsq", [P, T], f32, kind="ExternalOutput")
    colsum_dram = nc.dram_tensor("colsum", [1, D], f32, kind="ExternalOutput")

    with tile.TileContext(nc) as tc:
        with (
            tc.tile_pool(name="xpool", bufs=1) as xpool,
            tc.tile_pool(name="scr_a", bufs=2) as scr_a,
            tc.tile_pool(name="scr_b", bufs=2) as scr_b,
            tc.tile_pool(name="pairs", bufs=4) as pairs,
            tc.tile_pool(name="stats", bufs=1) as stats,
            tc.tile_pool(name="psum", bufs=1, space=bass.MemorySpace.PSUM) as psum_pool,
        ):
            X = xpool.tile([P, T, D], f32)
            # partition p <- DRAM rows p*T .. p*T+T-1 (contiguous 16KB)
            x_r = x_dram[:].rearrange("(p t) d -> p t d", p=P)

            rowsq = stats.tile([P, T], f32)
            ps = psum_pool.tile([1, D], f32)
            onesb = nc.const_aps.tensor(1.0, [P, 1], bf16)

            # 4 chunks of 2 row-tiles alternating between the two HWDGE
            # rings so two transfers are in flight and each chunk's
            # completion semaphore gates only its own tiles' compute.
            for c in range(NCHUNKS):
                sl = slice(c * TPC, (c + 1) * TPC)
                eng = nc.scalar if c % 2 == 0 else nc.sync
                eng.dma_start(X[:, sl, :], x_r[:, sl, :])

            def act_square(t, col):
                xsq_a = scr_a.tile([P, D], f32, tag="xsq_a")
                nc.scalar.activation(
                    xsq_a[:],
                    X[:, t, :],
                    mybir.ActivationFunctionType.Square,
                    accum_out=rowsq[:, col : col + 1],
                )

            def stt_square(eng, t, col, tag, pool):
                xsq = pool.tile([P, D], f32, tag=tag)
                eng.scalar_tensor_tensor(
                    out=xsq[:],
                    in0=X[:, t, :],
                    scalar=1.0,
                    in1=X[:, t, :],
                    op0=mybir.AluOpType.mult,
                    op1=mybir.AluOpType.mult,
                    accum_out=rowsq[:, col : col + 1],
                )

            def pair_mm(c, start, stop):
                pair = pairs.tile([P, D], bf16, tag="pair")
                nc.vector.tensor_add(pair[:], X[:, 2 * c, :], X[:, 2 * c + 1, :])
                nc.tensor.matmul(ps[:], onesb, pair[:], start=start, stop=stop)

            # DVE runs all four pairs as their chunks land (deferred squares
            # queue behind them) so the PSUM accumulation finishes as early
            # as possible; ACT carries five squares plus the PSUM copy.
            pair_mm(0, True, False)
            stt_square(nc.vector, 1, 4, "xsq_b", scr_b)
            act_square(0, 0)
            act_square(2, 1)
            pair_mm(1, False, False)
            pair_mm(2, False, False)
            pair_mm(3, False, True)
            stt_square(nc.vector, 3, 5, "xsq_b", scr_b)
            act_square(4, 2)
            act_square(6, 3)
            act_square(5, 6)
            stt_square(nc.vector, 7, 7, "xsq_b", scr_b)

            colsum = stats.tile([1, D], f32)
            nc.scalar.copy(colsum[:], ps[:])

            nc.sync.dma_start(rowsq_dram[:], rowsq[:])
            nc.scalar.dma_start(colsum_dram[:], colsum[:])

    nc.compile()
    return nc


def get_nc():
    if not _nc_cache:
        _nc_cache.append(_build_nc())
    return _nc_cache[0]


def combine_partials(rowsq_parts, colsum_parts):
    """rowsq_parts: per-core (P, T//2) row-squared-norm arrays; colsum_parts:
    per-core (1, D) column sums -> scalar loss.  Row order is irrelevant
    for sum/max, so no reindexing is needed."""
    S = 0.0
    maxsq = -np.inf
    for r in rowsq_parts:
        S += r.sum(dtype=np.float64)
        maxsq = max(maxsq, float(r.max()))
    s = np.zeros(D, dtype=np.float64)
    for cs in colsum_parts:
        s += cs.reshape(-1).astype(np.float64)
    count = N * (N - 1) // 2
    loss = (N * S - s @ s) / (np.sqrt(maxsq) * count)
    return np.float32(loss)


def kernel(x):
    from concourse.bass_utils import run_bass_kernel_spmd

    x = np.ascontiguousarray(np.asarray(x), dtype=np.float32)
    assert x.shape == (N, D), x.shape
    nc = get_nc()
    in_maps = [{"x_blk": x[c * ROWS : (c + 1) * ROWS]} for c in range(NCORES)]
    res = run_bass_kernel_spmd(nc, in_maps, list(range(NCORES)))
    rowsq_parts = [r["rowsq"] for r in res.results]
    colsum_parts = [r["colsum"] for r in res.results]
    return combine_partials(rowsq_parts, colsum_parts)



# revision 2
# speedup vs baseline: 1.0576x; 1.0576x over previous
"""Trainium2 Bass kernel for nn_NormalizedDistanceLoss.

Math: for x in R^{N x D}, with sq_i = ||x_i||^2, the strict-upper-triangle
sum of pairwise squared distances collapses algebraically:

    sum_{i<j} (sq_i + sq_j - 2 x_i.x_j) = N * S - ||s||^2

where S = sum_i sq_i and s = sum_i x_i (column sums).  So the loss

    loss = sum_masked_dist / (sqrt(max_i sq_i) * N(N-1)/2)

needs only one pass over x: per-row squared norms (for S and the max)
and column sums (for s).  Each of the 8 cores reduces its 1024-row block;
the host combines tiny per-core partials (a few KB per core).

Per-core device kernel (block = 1024 x 512 f32), raw bass (no TileContext,
manual semaphores) to minimize fixed framework overhead:
  - SBUF X[128, 8, 512]: partition p holds DRAM rows p*8..p*8+7 (16KB
    contiguous per partition).
  - Input DMA: 3 chunks sized against the ~27ns/descriptor per-queue HWDGE
    feed rate: tiles 4-5 on the SP ring set (4KB descs, lands first),
    tiles 0-3 on the Act ring set (8KB descs), tiles 6-7 on SP (lands
    last, carrying only one pair + two squares of tail work).
  - Row squared norms: fused square+row-accumulate; ACT (Square
    activation + accum_out) and DVE (scalar_tensor_tensor + accum_out)
    split the 8 tiles 4/4.
  - Column sums: DVE adds tile pairs into bf16 tiles; the otherwise-idle
    PE contracts partitions with a ones-vector matmul accumulated in one
    PSUM bank; ACT copies PSUM->SBUF.  bf16 pair rounding perturbs the
    loss ~1e-8 relative.
  - The output DMAs are issued but NOT waited on: the NEFF epilogue
    drains the DMA queues, so the output flight overlaps the fixed
    semaphore-clear teardown.
"""

import sys

if "/opt/trn_rl_repo" not in sys.path:
    sys.path.insert(0, "/opt/trn_rl_repo")

import numpy as np

from concourse import bacc, mybir

N = 8192
D = 512
NCORES = 8
ROWS = N // NCORES  # 1024 rows per core
P = 128
T = ROWS // P  # 8 row-tiles of 512

_nc_cache = []


def _build_nc():
    f32 = mybir.dt.float32
    bf16 = mybir.dt.bfloat16
    nc = bacc.Bacc(
        "TRN2",
        target_bir_lowering=False,
        debug=False,
        num_devices=NCORES,
    )
    x_dram = nc.dram_tensor("x_blk", [ROWS, D], f32, kind="ExternalInput")
    rowsq_dram = nc.dram_tensor("rowsq", [P, T], f32, kind="ExternalOutput")
    colsum_dram = nc.dram_tensor("colsum", [1, D], f32, kind="ExternalOutput")

    X = nc.alloc_sbuf_tensor("X", [P, T, D], f32)
    rowsq = nc.alloc_sbuf_tensor("rowsq_sb", [P, T], f32)
    xsq_a = nc.alloc_sbuf_tensor("xsq_a", [P, D], f32)
    xsq_b = nc.alloc_sbuf_tensor("xsq_b", [P, D], f32)
    pairs = [nc.alloc_sbuf_tensor(f"pair{k}", [P, D], bf16) for k in range(4)]
    colsum = nc.alloc_sbuf_tensor("colsum_sb", [1, D], f32)
    ps = nc.alloc_psum_tensor("ps", [1, D], f32)
    onesb = nc.const_aps.tensor(1.0, [P, 1], bf16)

    sA = nc.alloc_semaphore("sA")  # scalar-queue chunk (tiles 0-3)
    sB = nc.alloc_semaphore("sB")  # sync-queue chunks (tiles 4-5, 6-7)
    sSq = nc.alloc_semaphore("sSq")  # row-sum landings (+1 x8)
    sPr = nc.alloc_semaphore("sPr")  # pair tensor_adds (+1 x4)
    sMM = nc.alloc_semaphore("sMM")  # last matmul
    sC = nc.alloc_semaphore("sC")  # PSUM copy done
    sOut = nc.alloc_semaphore("sOut")  # output DMA completions; never waited

    x_r = x_dram[:].rearrange("(p t) d -> p t d", p=P)

    # input DMAs
    nc.sync.dma_start(X[:, 4:6, :], x_r[:, 4:6, :]).then_inc(sB, 16)
    nc.scalar.dma_start(X[:, 0:4, :], x_r[:, 0:4, :]).then_inc(sA, 16)
    nc.sync.dma_start(X[:, 6:8, :], x_r[:, 6:8, :]).then_inc(sB, 16)

    def act_square(t, col):
        nc.scalar.activation(
            xsq_a[:],
            X[:, t, :],
            mybir.ActivationFunctionType.Square,
            accum_out=rowsq[:, col : col + 1],
        ).then_inc(sSq, 1)

    def dve_square(t, col):
        nc.vector.scalar_tensor_tensor(
            out=xsq_b[:],
            in0=X[:, t, :],
            scalar=1.0,
            in1=X[:, t, :],
            op0=mybir.AluOpType.mult,
            op1=mybir.AluOpType.mult,
            accum_out=rowsq[:, col : col + 1],
        ).then_inc(sSq, 1)

    # DVE: pair adds first (feeding PE), stt squares behind
    nc.vector.wait_ge(sB, 16)
    nc.vector.tensor_add(pairs[0][:], X[:, 4, :], X[:, 5, :]).then_inc(sPr, 1)
    dve_square(5, 1)
    nc.vector.wait_ge(sA, 16)
    nc.vector.tensor_add(pairs[1][:], X[:, 0, :], X[:, 1, :]).then_inc(sPr, 1)
    nc.vector.tensor_add(pairs[2][:], X[:, 2, :], X[:, 3, :]).then_inc(sPr, 1)
    dve_square(1, 3)
    nc.vector.wait_ge(sB, 32)
    nc.vector.tensor_add(pairs[3][:], X[:, 6, :], X[:, 7, :]).then_inc(sPr, 1)
    dve_square(3, 5)
    dve_square(7, 7)

    # ACT: squares + PSUM copy + colsum DMA
    nc.scalar.wait_ge(sB, 16)
    act_square(4, 0)
    nc.scalar.wait_ge(sA, 16)
    act_square(0, 2)
    act_square(2, 4)
    nc.scalar.wait_ge(sB, 32)
    act_square(6, 6)
    nc.scalar.wait_ge(sMM, 1)
    nc.scalar.copy(colsum[:], ps[:]).then_inc(sC, 1)
    nc.scalar.wait_ge(sC, 1)
    nc.scalar.dma_start(colsum_dram[:], colsum[:]).then_inc(sOut, 16)

    # PE: ones-contraction of pair tiles accumulated in PSUM
    for k in range(4):
        nc.tensor.wait_ge(sPr, k + 1)
        mm = nc.tensor.matmul(ps[:], onesb, pairs[k][:], start=(k == 0), stop=(k == 3))
    mm.then_inc(sMM, 1)

    # SP: rowsq output once all 8 squares landed
    nc.sync.wait_ge(sSq, 8)
    nc.sync.dma_start(rowsq_dram[:], rowsq[:]).then_inc(sOut, 16)

    nc.compile()
    return nc


def get_nc():
    if not _nc_cache:
        _nc_cache.append(_build_nc())
    return _nc_cache[0]


def combine_partials(rowsq_parts, colsum_parts):
    """rowsq_parts: per-core (P, T) row-squared-norm arrays; colsum_parts:
    per-core (1, D) column sums -> scalar loss.  Row order is irrelevant
    for sum/max, so no reindexing is needed."""
    S = 0.0
    maxsq = -np.inf
    for r in rowsq_parts:
        S += r.sum(dtype=np.float64)
        maxsq = max(maxsq, float(r.max()))
    s = np.zeros(D, dtype=np.float64)
    for cs in colsum_parts:
        s += cs.reshape(-1).astype(np.float64)
    count = N * (N - 1) // 2
    loss = (N * S - s @ s) / (np.sqrt(maxsq) * count)
    return np.float32(loss)


def kernel(x):
    from concourse.bass_utils import run_bass_kernel_spmd

    x = np.ascontiguousarray(np.asarray(x), dtype=np.float32)
    assert x.shape == (N, D), x.shape
    nc = get_nc()
    in_maps = [{"x_blk": x[c * ROWS : (c + 1) * ROWS]} for c in range(NCORES)]
    res = run_bass_kernel_spmd(nc, in_maps, list(range(NCORES)))
    rowsq_parts = [r["rowsq"] for r in res.results]
    colsum_parts = [r["colsum"] for r in res.results]
    return combine_partials(rowsq_parts, colsum_parts)


# revision 3
# speedup vs baseline: 1.1037x; 1.0435x over previous
"""Trainium2 Bass kernel for nn_NormalizedDistanceLoss.

Math: for x in R^{N x D}, with sq_i = ||x_i||^2, the strict-upper-triangle
sum of pairwise squared distances collapses algebraically:

    sum_{i<j} (sq_i + sq_j - 2 x_i.x_j) = N * S - ||s||^2

where S = sum_i sq_i and s = sum_i x_i (column sums).  So the loss

    loss = sum_masked_dist / (sqrt(max_i sq_i) * N(N-1)/2)

needs only one pass over x: per-row squared norms (for S and the max)
and column sums (for s).  Each of the 8 cores reduces its 1024-row block;
the host combines tiny per-core partials (a few KB per core).

Per-core device kernel (block = 1024 x 512 f32), raw bass (no TileContext,
manual semaphores) to minimize fixed framework overhead:
  - SBUF X[128, 8, 512]: partition p holds DRAM rows p*8..p*8+7 (16KB
    contiguous per partition).
  - Input DMA: each HWDGE queue generates descriptors at only ~27ns each,
    so chunk layout balances descriptor counts (128 per chunk): tiles 4-5
    on the SP ring set (4KB descriptors, lands first), then tiles 0-3
    (8KB descriptors) and tiles 6-7 (4KB) both on the Act ring set.  The
    last chunk carries only one pair-add and two squares of tail work.
  - Row squared norms: fused square+row-accumulate; ACT (Square
    activation + accum_out) and DVE (scalar_tensor_tensor + accum_out)
    split the 8 tiles 4/4.
  - Column sums: DVE adds tile pairs into bf16 tiles; the otherwise-idle
    PE contracts the 128 partitions with a ones-vector matmul accumulated
    in one PSUM bank; ACT copies PSUM->SBUF.  bf16 pair rounding perturbs
    the loss ~1e-8 relative.
  - The output DMAs are issued but NOT waited on: the NEFF epilogue
    drains the DMA queues, so the output flight overlaps the fixed
    semaphore-clear teardown (~6us of EVENT_SEMAPHORE clears that
    dominates the non-body time).
"""

import sys

if "/opt/trn_rl_repo" not in sys.path:
    sys.path.insert(0, "/opt/trn_rl_repo")

import numpy as np

from concourse import bacc, mybir

N = 8192
D = 512
NCORES = 8
ROWS = N // NCORES  # 1024 rows per core
P = 128
T = ROWS // P  # 8 row-tiles of 512

# input chunks in expected-completion order: (engine, tile_lo, tile_hi)
CHUNKS = [("s", 4, 6), ("a", 0, 4), ("a", 6, 8)]

_nc_cache = []


def _build_nc():
    f32 = mybir.dt.float32
    bf16 = mybir.dt.bfloat16
    nc = bacc.Bacc(
        "TRN2",
        target_bir_lowering=False,
        debug=False,
        num_devices=NCORES,
    )
    x_dram = nc.dram_tensor("x_blk", [ROWS, D], f32, kind="ExternalInput")
    rowsq_dram = nc.dram_tensor("rowsq", [P, T], f32, kind="ExternalOutput")
    colsum_dram = nc.dram_tensor("colsum", [1, D], f32, kind="ExternalOutput")

    X = nc.alloc_sbuf_tensor("X", [P, T, D], f32)
    rowsq = nc.alloc_sbuf_tensor("rowsq_sb", [P, T], f32)
    xsq_a = nc.alloc_sbuf_tensor("xsq_a", [P, D], f32)
    xsq_b = nc.alloc_sbuf_tensor("xsq_b", [P, D], f32)
    pairs = [nc.alloc_sbuf_tensor(f"pair{k}", [P, D], bf16) for k in range(4)]
    colsum = nc.alloc_sbuf_tensor("colsum_sb", [1, D], f32)
    ps = nc.alloc_psum_tensor("ps", [1, D], f32)
    onesb = nc.const_aps.tensor(1.0, [P, 1], bf16)

    engs = {"a": nc.scalar, "s": nc.sync}

    sSq = nc.alloc_semaphore("sSq")  # row-sum landings (+1 x8)
    sPr = nc.alloc_semaphore("sPr")  # pair tensor_adds (+1 x4)
    sMM = nc.alloc_semaphore("sMM")  # last matmul done
    sC = nc.alloc_semaphore("sC")  # PSUM copy done
    sOut = nc.alloc_semaphore("sOut")  # output DMA completions; never waited
    qsems = {e: nc.alloc_semaphore(f"sD_{e}") for e in ("a", "s")}

    x_r = x_dram[:].rearrange("(p t) d -> p t d", p=P)

    for eng, lo, hi in CHUNKS:
        engs[eng].dma_start(X[:, lo:hi, :], x_r[:, lo:hi, :]).then_inc(qsems[eng], 16)

    def act_square(t, col):
        nc.scalar.activation(
            xsq_a[:],
            X[:, t, :],
            mybir.ActivationFunctionType.Square,
            accum_out=rowsq[:, col : col + 1],
        ).then_inc(sSq, 1)

    def dve_square(t, col):
        nc.vector.scalar_tensor_tensor(
            out=xsq_b[:],
            in0=X[:, t, :],
            scalar=1.0,
            in1=X[:, t, :],
            op0=mybir.AluOpType.mult,
            op1=mybir.AluOpType.mult,
            accum_out=rowsq[:, col : col + 1],
        ).then_inc(sSq, 1)

    # per chunk: DVE pair-adds first (feeding PE), squares behind,
    # alternating ACT/DVE
    col = 0
    npair = 0
    alt = 0
    seen = {"a": 0, "s": 0}
    for eng, lo, hi in CHUNKS:
        seen[eng] += 16
        nc.vector.wait_ge(qsems[eng], seen[eng])
        nc.scalar.wait_ge(qsems[eng], seen[eng])
        for a in range(lo, hi, 2):
            nc.vector.tensor_add(pairs[npair][:], X[:, a, :], X[:, a + 1, :]).then_inc(
                sPr, 1
            )
            npair += 1
        for t in range(lo, hi):
            if alt % 2 == 0:
                act_square(t, col)
            else:
                dve_square(t, col)
            alt += 1
            col += 1

    # PE: ones-contraction of pair tiles accumulated in one PSUM bank
    for k in range(4):
        nc.tensor.wait_ge(sPr, k + 1)
        mm = nc.tensor.matmul(ps[:], onesb, pairs[k][:], start=(k == 0), stop=(k == 3))
    mm.then_inc(sMM, 1)

    # ACT: PSUM copy + colsum output
    nc.scalar.wait_ge(sMM, 1)
    nc.scalar.copy(colsum[:], ps[:]).then_inc(sC, 1)
    nc.scalar.wait_ge(sC, 1)
    nc.scalar.dma_start(colsum_dram[:], colsum[:]).then_inc(sOut, 16)

    # SP: rowsq output once all 8 squares landed
    nc.sync.wait_ge(sSq, 8)
    nc.sync.dma_start(rowsq_dram[:], rowsq[:]).then_inc(sOut, 16)

    nc.compile()
    return nc


def get_nc():
    if not _nc_cache:
        _nc_cache.append(_build_nc())
    return _nc_cache[0]


def combine_partials(rowsq_parts, colsum_parts):
    """rowsq_parts: per-core (P, T) row-squared-norm arrays; colsum_parts:
    per-core (1, D) column sums -> scalar loss.  Row order is irrelevant
    for sum/max, so no reindexing is needed."""
    S = 0.0
    maxsq = -np.inf
    for r in rowsq_parts:
        S += r.sum(dtype=np.float64)
        maxsq = max(maxsq, float(r.max()))
    s = np.zeros(D, dtype=np.float64)
    for cs in colsum_parts:
        s += cs.reshape(-1).astype(np.float64)
    count = N * (N - 1) // 2
    loss = (N * S - s @ s) / (np.sqrt(maxsq) * count)
    return np.float32(loss)


def kernel(x):
    from concourse.bass_utils import run_bass_kernel_spmd

    x = np.ascontiguousarray(np.asarray(x), dtype=np.float32)
    assert x.shape == (N, D), x.shape
    nc = get_nc()
    in_maps = [{"x_blk": x[c * ROWS : (c + 1) * ROWS]} for c in range(NCORES)]
    res = run_bass_kernel_spmd(nc, in_maps, list(range(NCORES)))
    rowsq_parts = [r["rowsq"] for r in res.results]
    colsum_parts = [r["colsum"] for r in res.results]
    return combine_partials(rowsq_parts, colsum_parts)
